# revision 2
# baseline (speedup 1.0000x reference)
"""Longformer attention v2: 8 cores = 2 query-shards x 4 head-pairs.

Per core (shard sh, head pair g -> heads 2g, 2g+1):
  - queries Q = [2048*sh, 2048*sh+2048), all masks/pads data-driven so one
    program serves both shards.
  - kv tiles: 21 tiles of 128 keys = band [base-256, base+2304) zero-padded
    outside [0,4096) + one "gtile" (keys 0:128) for global-key columns.
  - logits [k, q] per (head, 256-q-tile): 6 band tiles + gtile; triangular /
    global masks added into PSUM as bias matmuls (lhsT=I, rhs=bias data);
    pad keys contribute exp(0)=1, subtracted later via host-computed n_pad.
  - AV via v_nat (PE-transposed) with ones column -> denominators row.
  - out-projection PER HEAD (c64), scaled by 1/denominator per query row;
    host sums 8 partial outputs per shard.
  - global rows (q < ng): Z-trick: logits = (q_g W_kg^T /8) @ x_kv^T computed
    from x_kvT directly; Y = Eg^T @ x_kv_nat returned; host applies W_vg,
    normalizes, and out-projects the ng rows.
Biases: b_k_sw, b_k_g drop (softmax row-constant); b_v_sw/b_v_g/b_out/b_q_g
  handled on host; b_q_sw must be zero (asserted).
"""
import os
import sys
import functools

for _p in ("/opt/trn_rl_repo",):
    if os.path.isdir(_p) and _p not in sys.path:
        sys.path.insert(0, _p)

import numpy as np
import ml_dtypes

import concourse.bass as bass
import concourse.tile as tile
from concourse import bacc, mybir
from concourse.bass_utils import run_bass_kernel_spmd

S, F, HD, H = 4096, 512, 64, 8
QC = 2048          # local queries per core
NQT = 8            # 256-query tiles
QT = 256
NT = 21            # kv tiles (20 band + gtile)
KVC = NT * 128     # 2688
NBLK = 16          # 128-query blocks
N_CORES = 8
NEG = -240.0       # mask bias (exp(NEG/8) ~ 9e-14)

F32 = mybir.dt.float32
BF16 = mybir.dt.bfloat16
BF = ml_dtypes.bfloat16

MORDER = [0, 1, 4, 5, 2, 3]                       # masked tiles first
POS = {0: 0, 1: 256, 4: 512, 5: 768, 2: 1024, 3: 1280}


def _build_program(dbg=False):
    nc = bacc.Bacc("TRN2", target_bir_lowering=False, debug=False,
                   num_devices=N_CORES)
    d = {}

    def inp(name, shape, dt=BF16):
        d[name] = nc.dram_tensor(name, shape, dt, kind="ExternalInput").ap()

    inp("xqT", [F, QC])
    inp("xkvT", [F, KVC])
    inp("xkvn", [QC, F])
    inp("zt2", [F, 128])
    inp("wq", [F, 128]); inp("wk", [F, 128]); inp("wv", [F, 128])
    inp("woA", [64, F]); inp("wo2", [128, F])
    inp("identb", [128, 128])
    inp("bias01", [128, 512]); inp("bias45", [128, 512])
    inp("maskg", [8, 128, 256])
    inp("npad", [128, 32], F32)
    outA = nc.dram_tensor("outA", [QC, F], BF16, kind="ExternalOutput").ap()
    outB = nc.dram_tensor("outB", [QC, F], BF16, kind="ExternalOutput").ap()
    outY = nc.dram_tensor("outY", [128, F], F32, kind="ExternalOutput").ap()
    outG = nc.dram_tensor("outG", [128, 1], F32, kind="ExternalOutput").ap()
    if dbg:
        dqT = nc.dram_tensor("dqT", [128, QC], BF16, kind="ExternalOutput").ap()
        dkT = nc.dram_tensor("dkT", [128, KVC], BF16, kind="ExternalOutput").ap()
        dvn = nc.dram_tensor("dvn", [128, NT, 130], BF16, kind="ExternalOutput").ap()
        dxA = nc.dram_tensor("dxA", [65, QC], BF16, kind="ExternalOutput").ap()
        dxB = nc.dram_tensor("dxB", [65, QC], BF16, kind="ExternalOutput").ap()
        dxsB = nc.dram_tensor("dxsB", [128, QC], BF16, kind="ExternalOutput").ap()
        drc = nc.dram_tensor("drc", [128, 32], F32, kind="ExternalOutput").ap()
        dE0 = nc.dram_tensor("dE0", [128, 3, 512], BF16, kind="ExternalOutput").ap()
        dE1 = nc.dram_tensor("dE1", [128, 3, 512], BF16, kind="ExternalOutput").ap()
        dg0 = nc.dram_tensor("dg0", [128, 256], BF16, kind="ExternalOutput").ap()

    with tile.TileContext(nc) as tc:
        with tc.tile_pool(name="const", bufs=1) as cp:
            # ---- persistent sbuf ----
            xqT = cp.tile([128, 4, QC], BF16, tag="xqT")
            xkvT = cp.tile([128, 4, KVC], BF16, tag="xkvT")
            xkvn = cp.tile([128, 16, F], BF16, tag="xkvn")
            zt2 = cp.tile([128, 4, 128], BF16, tag="zt2")
            wq = cp.tile([128, 4, 128], BF16, tag="wq")
            wk = cp.tile([128, 4, 128], BF16, tag="wk")
            wv = cp.tile([128, 4, 128], BF16, tag="wv")
            woA = cp.tile([64, F], BF16, tag="woA")
            wo2 = cp.tile([128, F], BF16, tag="wo2")
            identb = cp.tile([128, 128], BF16, tag="identb")
            bias01 = cp.tile([128, 512], BF16, tag="bias01")
            bias45 = cp.tile([128, 512], BF16, tag="bias45")
            maskg = cp.tile([128, 8, 256], BF16, tag="maskg")
            npad = cp.tile([128, 32], F32, tag="npad")
            for n, t in (("xqT", xqT), ("xkvT", xkvT), ("zt2", zt2),
                         ("wq", wq), ("wk", wk), ("wv", wv)):
                nc.sync.dma_start(t[:], d[n].rearrange("(c p) s -> p c s", p=128))
            nc.sync.dma_start(xkvn[:], d["xkvn"].rearrange("(t p) f -> p t f", p=128))
            nc.sync.dma_start(woA[:], d["woA"][:])
            nc.sync.dma_start(wo2[:], d["wo2"][:])
            nc.sync.dma_start(identb[:], d["identb"][:])
            nc.sync.dma_start(bias01[:], d["bias01"][:])
            nc.sync.dma_start(bias45[:], d["bias45"][:])
            nc.sync.dma_start(maskg[:], d["maskg"].rearrange("q p c -> p q c"))
            nc.sync.dma_start(npad[:], d["npad"][:])

            qT = cp.tile([128, QC], BF16, tag="qT")
            kT = cp.tile([128, KVC], BF16, tag="kT")
            vTt = cp.tile([128, KVC], BF16, tag="vTt")
            v_nat = cp.tile([128, NT, 130], BF16, tag="v_nat")
            xA = cp.tile([65, QC], BF16, tag="xA")
            xB = cp.tile([65, QC], BF16, tag="xB")
            xsB = cp.tile([128, QC], BF16, tag="xsB")
            Egq = cp.tile([128, QC, ], BF16, tag="Egq")
            Eg_nat = cp.tile([128, 16, 128], BF16, tag="Eg_nat")
            gacc = cp.tile([128, 4], F32, tag="gacc")
            gdens = cp.tile([128, 1], F32, tag="gdens")
            Ysb = cp.tile([128, F], F32, tag="Ysb")
            rc = cp.tile([128, 32], F32, tag="rc")
            rc0 = cp.tile([128, 32], F32, tag="rc0")
            oneb = cp.tile([128, 1], BF16, tag="oneb")
            nc.vector.memset(oneb[:], 1.0)
            nc.vector.memset(v_nat[:, :, 64:65], 1.0)
            nc.vector.memset(v_nat[:, :, 129:130], 1.0)

            # ================= Phase A: projections =================
            with (
                tc.tile_pool(name="pa", bufs=4, space="PSUM") as pap,
                tc.tile_pool(name="ptr", bufs=2, space="PSUM") as ptrp,
            ):
                for sc in range(4):            # q-proj, 512-col chunks
                    pq = pap.tile([128, 512], F32, tag="pa")
                    for fc in range(4):
                        nc.tensor.matmul(pq[:], wq[:, fc, :],
                                         xqT[:, fc, sc * 512:(sc + 1) * 512],
                                         start=(fc == 0), stop=(fc == 3))
                    nc.vector.tensor_copy(qT[:, sc * 512:(sc + 1) * 512], pq[:])
                for sc in range(6):            # k/v-proj, 448-col chunks
                    s0 = sc * 448
                    pk = pap.tile([128, 512], F32, tag="pa")
                    for fc in range(4):
                        nc.tensor.matmul(pk[:, 0:448], wk[:, fc, :],
                                         xkvT[:, fc, s0:s0 + 448],
                                         start=(fc == 0), stop=(fc == 3))
                    nc.vector.tensor_copy(kT[:, s0:s0 + 448], pk[:, 0:448])
                    pv = pap.tile([128, 512], F32, tag="pa")
                    for fc in range(4):
                        nc.tensor.matmul(pv[:, 0:448], wv[:, fc, :],
                                         xkvT[:, fc, s0:s0 + 448],
                                         start=(fc == 0), stop=(fc == 3))
                    nc.vector.tensor_copy(vTt[:, s0:s0 + 448], pv[:, 0:448])
                # v transposes -> v_nat [k, (vA|1|vB|1)]
                for tg in range(6):
                    n = min(4, NT - tg * 4)
                    ptr = ptrp.tile([128, 4, 128], BF16, tag="tr")
                    for j in range(n):
                        t = tg * 4 + j
                        nc.tensor.transpose(ptr[:, j, :],
                                            vTt[:, t * 128:(t + 1) * 128],
                                            identb[:])
                    nc.vector.tensor_copy(v_nat[:, tg * 4:tg * 4 + n, 0:64],
                                          ptr[:, 0:n, 0:64])
                    nc.vector.tensor_copy(v_nat[:, tg * 4:tg * 4 + n, 65:129],
                                          ptr[:, 0:n, 64:128])

            # ================= Phase B: global rows =================
            with (
                tc.tile_pool(name="pz", bufs=1, space="PSUM") as pzp,
                tc.tile_pool(name="pg2", bufs=2, space="PSUM") as pg2p,
                tc.tile_pool(name="pY", bufs=1, space="PSUM") as pYp,
            ):
                pz = pzp.tile([128, 4, 512], F32, tag="pz")
                for kc in range(4):
                    for fc in range(4):
                        nc.tensor.matmul(
                            pz[:, kc, :], zt2[:, fc, :],
                            xkvT[:, fc, 256 + kc * 512:256 + (kc + 1) * 512],
                            start=(fc == 0), stop=(fc == 3))
                    nc.scalar.activation(Egq[:, kc * 512:(kc + 1) * 512],
                                         pz[:, kc, :],
                                         mybir.ActivationFunctionType.Exp,
                                         scale=1.0, accum_out=gacc[:, kc:kc + 1])
                nc.vector.tensor_reduce(gdens[:], gacc[:],
                                        mybir.AxisListType.X,
                                        mybir.AluOpType.add)
                for tg in range(4):
                    pt2 = pg2p.tile([128, 4, 128], BF16, tag="tr2")
                    for j in range(4):
                        t = tg * 4 + j
                        nc.tensor.transpose(pt2[:, j, :],
                                            Egq[:, t * 128:(t + 1) * 128],
                                            identb[:])
                    nc.vector.tensor_copy(Eg_nat[:, tg * 4:tg * 4 + 4, :], pt2[:])
                pY = pYp.tile([128, F], F32, tag="pY")
                for t in range(16):
                    nc.tensor.matmul(pY[:], Eg_nat[:, t, :], xkvn[:, t, :],
                                     start=(t == 0), stop=(t == 15))
                nc.vector.tensor_copy(Ysb[:], pY[:])
                nc.sync.dma_start(outY[:], Ysb[:])
                nc.sync.dma_start(outG[:], gdens[:])

            # ================= Phase C: sliding-window =================
            esb_all = [[None, None] for _ in range(NQT)]
            gesb_all = [[None, None] for _ in range(NQT)]
            with (
                tc.tile_pool(name="pe", bufs=2, space="PSUM") as pep,
                tc.tile_pool(name="pg", bufs=1, space="PSUM") as pgp,
                tc.tile_pool(name="px", bufs=1, space="PSUM") as pxp,
                tc.tile_pool(name="esb", bufs=4) as esbp,
                tc.tile_pool(name="gesb", bufs=4) as gesbp,
            ):
                for qt in range(NQT + 1):
                    if qt < NQT:
                        EA = pep.tile([128, 3, 512], F32, tag="E")
                        EB = pep.tile([128, 3, 512], F32, tag="E")
                        E2 = [EA, EB]
                        gE = pgp.tile([128, 2, 256], F32, tag="g")
                        q0 = qt * QT
                        BIAS_RHS = {0: bias01[:, 0:256], 1: bias01[:, 256:512],
                                    4: bias45[:, 0:256], 5: bias45[:, 256:512]}
                        for m in MORDER:
                            t = 2 * qt + m
                            full = m in (2, 3)
                            for h in (0, 1):
                                p, o = POS[m] // 512, POS[m] % 512
                                nc.tensor.matmul(
                                    E2[h][:, p, o:o + 256],
                                    kT[h * 64:(h + 1) * 64, t * 128:(t + 1) * 128],
                                    qT[h * 64:(h + 1) * 64, q0:q0 + QT],
                                    start=True, stop=full,
                                    tile_position=(64 * h, 0),
                                    skip_group_check=True)
                            if not full:
                                for h in (0, 1):
                                    p, o = POS[m] // 512, POS[m] % 512
                                    nc.tensor.matmul(
                                        E2[h][:, p, o:o + 256],
                                        identb[:], BIAS_RHS[m],
                                        start=False, stop=True,
                                        skip_group_check=True)
                        for h in (0, 1):
                            nc.tensor.matmul(
                                gE[:, h, :],
                                kT[h * 64:(h + 1) * 64, 2560:2688],
                                qT[h * 64:(h + 1) * 64, q0:q0 + QT],
                                start=True, stop=False,
                                tile_position=(64 * h, 0),
                                skip_group_check=True)
                            nc.tensor.matmul(gE[:, h, :], identb[:],
                                             maskg[:, qt, :],
                                             start=False, stop=True,
                                             skip_group_check=True)
                            et = esbp.tile([128, 3, 512], BF16, tag="esb")
                            nc.scalar.activation(et[:], E2[h][:],
                                                 mybir.ActivationFunctionType.Exp,
                                                 scale=0.125)
                            gt = gesbp.tile([128, 256], BF16, tag="gesb")
                            nc.scalar.activation(gt[:], gE[:, h, :],
                                                 mybir.ActivationFunctionType.Exp,
                                                 scale=0.125)
                            esb_all[qt][h] = et
                            gesb_all[qt][h] = gt
                    if qt > 0:
                        qp = qt - 1
                        q0 = qp * QT
                        px = pxp.tile([128, 512], F32, tag="px")
                        for h in (0, 1):
                            for mi, m in enumerate(MORDER):
                                t = 2 * qp + m
                                p, o = POS[m] // 512, POS[m] % 512
                                nc.tensor.matmul(
                                    px[0:65, h * 256:h * 256 + 256],
                                    v_nat[:, t, h * 65:h * 65 + 65],
                                    esb_all[qp][h][:, p, o:o + 256],
                                    start=(mi == 0), stop=False)
                            nc.tensor.matmul(
                                px[0:65, h * 256:h * 256 + 256],
                                v_nat[:, 20, h * 65:h * 65 + 65],
                                gesb_all[qp][h][:],
                                start=False, stop=True)
                        nc.vector.tensor_copy(xA[:, q0:q0 + QT], px[0:65, 0:256])
                        nc.scalar.activation(xB[:, q0:q0 + QT],
                                             px[0:65, 256:512],
                                             mybir.ActivationFunctionType.Copy)

            # ================= Phase D: normalize + out-proj =================
            with (
                tc.tile_pool(name="pd", bufs=1, space="PSUM") as pdp,
                tc.tile_pool(name="po", bufs=4, space="PSUM") as pop,
                tc.tile_pool(name="osb", bufs=6) as osbp,
            ):
                nc.sync.dma_start(xsB[64:128, :], xB[0:64, :])
                dns = pdp.tile([128, 32], F32, tag="dns")
                for h, xh in ((0, xA), (1, xB)):
                    for blk in range(NBLK):
                        nc.tensor.matmul(dns[:, h * 16 + blk:h * 16 + blk + 1],
                                         xh[64:65, blk * 128:(blk + 1) * 128],
                                         oneb[64:65, :], start=True, stop=True)
                nc.vector.tensor_sub(rc0[:], dns[:], npad[:])
                nc.vector.reciprocal(rc[:], rc0[:])
                for blk in range(NBLK):
                    b0 = blk * 128
                    poA = pop.tile([128, F], F32, tag="po")
                    nc.tensor.matmul(poA[:], xA[0:64, b0:b0 + 128], woA[:],
                                     start=True, stop=True,
                                     tile_position=(0, 0))
                    poB = pop.tile([128, F], F32, tag="po")
                    nc.tensor.matmul(poB[:], xsB[64:128, b0:b0 + 128],
                                     wo2[64:128, :], start=True, stop=True,
                                     tile_position=(64, 0))
                    oA = osbp.tile([128, F], BF16, tag="oA")
                    nc.vector.tensor_scalar_mul(oA[:], poA[:],
                                                rc[:, blk:blk + 1])
                    oB = osbp.tile([128, F], BF16, tag="oB")
                    nc.scalar.activation(oB[:], poB[:],
                                         mybir.ActivationFunctionType.Copy,
                                         scale=rc[:, 16 + blk:16 + blk + 1])
                    nc.sync.dma_start(outA[b0:b0 + 128, :], oA[:])
                    nc.sync.dma_start(outB[b0:b0 + 128, :], oB[:])

    nc.compile()
    return nc


@functools.lru_cache(maxsize=2)
def _get_program(dbg=False):
    return _build_program(dbg)


def _masks_host(sh, ng):
    kk = np.arange(128)[:, None]
    qq = np.arange(256)[None, :]
    b01 = np.full((128, 512), NEG, np.float32)
    b01[:, 0:256][qq <= kk - 1] = 0.0
    b01[:, 256:512][qq <= kk + 127] = 0.0
    b45 = np.full((128, 512), NEG, np.float32)
    b45[:, 0:256][qq >= kk] = 0.0
    b45[:, 256:512][qq >= kk + 128] = 0.0
    mg = np.full((8, 128, 256), NEG, np.float32)
    if sh == 0:
        # qt0: globals fully inside band -> all masked; qt1: complement of the
        # m=0 triangle on global rows; qt>=2: plain global columns.
        mg[1][(kk < ng) & (qq >= kk)] = 0.0
        for q in range(2, 8):
            mg[q][(kk < ng) & (qq >= -1)] = 0.0
    else:
        for q in range(8):
            mg[q][(kk < ng) & (qq >= -1)] = 0.0
    return b01.astype(BF), b45.astype(BF), mg.astype(BF)


def kernel(inputs_q, inputs_kv, global_mask,
           w_q_sw, b_q_sw, w_k_sw, b_k_sw, w_v_sw, b_v_sw,
           w_q_g, b_q_g, w_k_g, b_k_g, w_v_g, b_v_g,
           w_out, b_out,
           _trace=False, _tmpdir=None):
    gm = np.asarray(global_mask[0]).astype(bool)
    ng = int(gm.sum())
    assert gm[:ng].all() and not gm[ng:].any(), "global_mask must be prefix"
    assert ng <= 64, "kernel specialized for ng <= 64"
    assert not np.any(np.asarray(b_q_sw)), "b_q_sw must be zero"

    xq = np.asarray(inputs_q[0], np.float32)
    xkv = np.asarray(inputs_kv[0], np.float32)
    xqT16 = xq.T.astype(BF)
    xkvT16 = xkv.T.astype(BF)
    xkv16 = xkv.astype(BF)

    wqs = np.asarray(w_q_sw, np.float32)
    wks = np.asarray(w_k_sw, np.float32)
    wvs = np.asarray(w_v_sw, np.float32)
    wqg = np.asarray(w_q_g, np.float32)
    wkg = np.asarray(w_k_g, np.float32)
    wvg = np.asarray(w_v_g, np.float32)
    wo = np.asarray(w_out, np.float32)

    ident = np.eye(128, dtype=np.float32).astype(BF)
    nc = _get_program(os.environ.get('KV2_DBG', '0') == '1')

    in_maps = []
    for core in range(N_CORES):
        sh, g = core // 4, core % 4
        base = 2048 * sh
        h0, h1 = 2 * g, 2 * g + 1

        xkvT_loc = np.zeros((512, KVC), np.float32)
        lo, hi = max(0, base - 256), min(S, base + 2304)
        d0 = lo - (base - 256)
        xkvT_loc[:, d0:d0 + (hi - lo)] = xkvT16[:, lo:hi].astype(np.float32)
        xkvT_loc[:, 2560:2688] = xkvT16[:, 0:128].astype(np.float32)

        qg = np.arange(base, base + QC)
        npad_v = np.maximum(0, 255 - qg) + np.maximum(0, qg - 3839)
        npad32 = np.zeros((128, 32), np.float32)
        for blk in range(16):
            col = npad_v[blk * 128:(blk + 1) * 128]
            npad32[:, blk] = col
            npad32[:, 16 + blk] = col

        zt2 = np.zeros((512, 128), np.float32)
        for j, h in enumerate((h0, h1)):
            qgh = xq[:ng] @ wqg[:, h] + np.asarray(b_q_g, np.float32)[h]
            zh = 0.125 * (qgh @ wkg[:, h].T)          # [ng, 512]
            zt2[:, j * 64:j * 64 + ng] = zh.T

        b01, b45, mg = _masks_host(sh, ng)
        in_maps.append({
            "xqT": np.ascontiguousarray(xqT16[:, base:base + QC]),
            "xkvT": xkvT_loc.astype(BF),
            "xkvn": np.ascontiguousarray(xkv16[base:base + QC]),
            "zt2": zt2.astype(BF),
            "wq": wqs[:, h0:h1 + 1].reshape(512, 128).astype(BF),
            "wk": wks[:, h0:h1 + 1].reshape(512, 128).astype(BF),
            "wv": wvs[:, h0:h1 + 1].reshape(512, 128).astype(BF),
            "woA": wo[h0].astype(BF),
            "wo2": wo[h0:h1 + 1].reshape(128, 512).astype(BF),
            "identb": ident, "bias01": b01, "bias45": b45,
            "maskg": mg, "npad": npad32,
        })

    res = run_bass_kernel_spmd(nc, in_maps, list(range(N_CORES)),
                               trace=_trace, tmpdir=_tmpdir)
    r = res.results

    out = np.zeros((S, F), np.float32)
    for sh in range(2):
        base = 2048 * sh
        acc = np.zeros((QC, F), np.float32)
        for g in range(4):
            c = sh * 4 + g
            acc += r[c]["outA"].astype(np.float32)
            acc += r[c]["outB"].astype(np.float32)
        out[base:base + QC] = acc
    bvs = np.asarray(b_v_sw, np.float32)       # [H, HD]
    out += (bvs.reshape(1, H * HD) @ wo.reshape(H * HD, F))
    out += np.asarray(b_out, np.float32)

    # global rows
    outg = np.zeros((ng, F), np.float32)
    bvg = np.asarray(b_v_g, np.float32)
    for g in range(4):
        c0, c1 = g, 4 + g
        for j, h in enumerate((2 * g, 2 * g + 1)):
            rows = slice(j * 64, j * 64 + ng)
            Yh = r[c0]["outY"][rows].astype(np.float32) + \
                r[c1]["outY"][rows].astype(np.float32)
            dh = r[c0]["outG"][rows, 0] + r[c1]["outG"][rows, 0]
            xg = (Yh @ wvg[:, h]) / dh[:, None] + bvg[h]
            outg += xg @ wo[h]
    outg += np.asarray(b_out, np.float32)
    out[0:ng] = outg

    kernel._last_results = res
    kernel._last_in_maps = in_maps
    return out[None].astype(np.float32)
